# revision 6
# baseline (speedup 1.0000x reference)
"""Two-layer GAT (DGL GATConv) forward on 8 Trainium2 NeuronCores.

Strategy (graph/data parallel, per sharding hint):
  - Destination nodes are partitioned into 8 contiguous blocks of 6250; each
    core owns the dst-segmented softmax + aggregation for its block.
  - Edges are sorted by dst on host and grouped into 128-dst "windows"; the
    weighted segment-sum over each window's edges is computed as a chain of
    PE matmuls against on-the-fly one-hot matrices accumulating in PSUM.
  - Node feature tables are replicated to every core's HBM; per-edge feature
    rows are fetched with per-tile indirect DMA gathers (128 rows each).
  - Per-edge attention logits (el[src], er[dst]) are small; the host permutes
    them into edge order between launches (pure data movement, same category
    as the sharding itself).
  - Three SPMD launches: A) dense projection x@W1 (+ el/er logits),
    B) layer-1 aggregation fused with the layer-2 projection h@W2,
    C) layer-2 aggregation + outputs. The host re-shards tables between
    launches.

Feature tables are bf16 (halves gather bytes, 2x DVE throughput); PSUM
accumulation is fp32. Edge softmax is computed without the segment-max
shift (exp arguments are bounded ~|8| for this model family, and
numerator/denominator share the same bf16-rounded exp values).
"""

import sys

if "/opt/trn_rl_repo" not in sys.path:
    sys.path.insert(0, "/opt/trn_rl_repo")

import numpy as np
import ml_dtypes

import concourse.bass as bass  # noqa: F401
import concourse.tile as tile
from concourse import bacc, mybir
from concourse.bass import IndirectOffsetOnAxis
from concourse.bass_utils import run_bass_kernel_spmd

BF = ml_dtypes.bfloat16
dt = mybir.dt

# ---- problem constants (hardcoded per spec) ----
N, E, F = 50000, 800000, 128
H1, D = 4, 64
C1 = H1 * D            # 256
CORES = 8
NB = N // CORES        # 6250 dst nodes per core
WIN = 128              # dst nodes per window
NW = (NB + WIN - 1) // WIN   # 49 windows per core
NBP = NW * WIN         # 6272 padded block rows
NEG_SLOPE = 0.2
PADROW = N             # gather-table pad row index
NEG_BIG = -1.0e30      # pad el -> exp() == 0.0 exactly
EPS = 1.0e-30


# ---------------------------------------------------------------------------
# host-side edge preprocessing (sharding)
# ---------------------------------------------------------------------------

def _preprocess_edges(src, dst):
    """Sort edges by dst, shard by dst block, window, and pad to tiles.

    Returns (K_w, per_core): K_w[w] = tiles in window w (shared across cores
    so the SPMD program is identical); per_core[c] holds offs_src (int32
    [128, T]) plus edge-ordered src/dst index arrays for host-side logit
    permutation, and dstrel (bf16 [128, T], pads = -1).
    """
    order = np.argsort(dst, kind="stable")
    src_s = src[order].astype(np.int64)
    dst_s = dst[order].astype(np.int64)

    bnds = np.empty((CORES, NW + 1), np.int64)
    for c in range(CORES):
        marks = c * NB + np.minimum(np.arange(NW + 1) * WIN, NB)
        bnds[c] = np.searchsorted(dst_s, marks, side="left")
    cnts = bnds[:, 1:] - bnds[:, :-1]
    K_w = np.maximum(1, (cnts.max(axis=0) + WIN - 1) // WIN).astype(np.int64)
    T = int(K_w.sum())
    tile_base = np.concatenate([[0], np.cumsum(K_w)])[:-1]

    per_core = []
    for c in range(CORES):
        osrc = np.full((T, WIN), PADROW, np.int64)    # pad -> zero pad row
        edst = np.zeros((T, WIN), np.int64)           # local dst (pads 0)
        drel = np.full((T, WIN), -1.0, np.float32)    # pad -> -1 (no match)
        pad = np.ones((T, WIN), bool)
        for w in range(NW):
            e0, e1 = bnds[c, w], bnds[c, w + 1]
            cnt = e1 - e0
            if cnt == 0:
                continue
            j = np.arange(cnt)
            t = tile_base[w] + j // WIN
            p = j % WIN
            osrc[t, p] = src_s[e0:e1]
            edst[t, p] = dst_s[e0:e1] - c * NB
            drel[t, p] = dst_s[e0:e1] - c * NB - w * WIN
            pad[t, p] = False
        per_core.append(dict(
            offs_src=np.ascontiguousarray(osrc.T).astype(np.int32),
            esrc=np.ascontiguousarray(osrc.T),        # [128, T] int64
            edst=np.ascontiguousarray(edst.T),
            is_pad=np.ascontiguousarray(pad.T),
            dstrel=np.ascontiguousarray(drel.T).astype(BF),
        ))
    return [int(k) for k in K_w], per_core


def _permute_logits(pc, el_full, er_blk):
    """Build per-edge el[src] / er[dst] arrays ([128, T, H] f32) on host.

    el_full: [N, H] (global); er_blk: [NB, H] (this core's dst block).
    Pad edges get el = NEG_BIG so exp(score) == 0.
    """
    Hh = el_full.shape[1]
    el_e = el_full[np.minimum(pc["esrc"], N - 1)].astype(np.float32)
    el_e[pc["is_pad"]] = NEG_BIG
    er_e = er_blk[pc["edst"]].astype(np.float32)
    return np.ascontiguousarray(el_e).reshape(128, -1, Hh), \
        np.ascontiguousarray(er_e).reshape(128, -1, Hh)


# ---------------------------------------------------------------------------
# shared device helpers
# ---------------------------------------------------------------------------

def _leaky_exp(nc, pool, es):
    """In-place es <- exp(leaky_relu(es, 0.2)). es is an f32 tile AP."""
    tmp = pool.tile(list(es.shape), dt.float32, tag="leakytmp")
    nc.vector.tensor_scalar_mul(tmp[:], es, NEG_SLOPE)
    nc.vector.tensor_tensor(out=es, in0=es, in1=tmp[:], op=mybir.AluOpType.max)
    nc.scalar.activation(es, es, mybir.ActivationFunctionType.Exp)


# ---------------------------------------------------------------------------
# launch A: feat1 = x @ W1, el1/er1 logits
# ---------------------------------------------------------------------------

def build_launch_A():
    nc = bacc.Bacc("TRN2", target_bir_lowering=False, debug=False,
                   num_devices=CORES)
    xT = nc.dram_tensor("xT", [F, NB], dt.float32, kind="ExternalInput")
    W1 = nc.dram_tensor("W1", [F, C1], dt.float32, kind="ExternalInput")
    W1Tst = nc.dram_tensor("W1Tst", [F, 2, F], dt.float32, kind="ExternalInput")
    alar = nc.dram_tensor("alar", [F, 2, 8], dt.float32, kind="ExternalInput")
    fb = nc.dram_tensor("featb", [NBP, C1], dt.bfloat16, kind="ExternalOutput")
    el = nc.dram_tensor("el1", [NBP, H1], dt.float32, kind="ExternalOutput")
    er = nc.dram_tensor("er1", [NBP, H1], dt.float32, kind="ExternalOutput")

    with tile.TileContext(nc) as tc:
        with tc.tile_pool(name="const", bufs=1) as cp, \
             tc.tile_pool(name="work", bufs=3) as wp, \
             tc.tile_pool(name="ps", bufs=2, space="PSUM") as pp:
            xT_sb = cp.tile([F, NB], dt.float32)
            nc.sync.dma_start(xT_sb[:], xT[:])
            rhs = cp.tile([F, C1 + 8], dt.float32)
            nc.sync.dma_start(rhs[:, 0:C1], W1[:])
            w1t_sb = cp.tile([F, 2, F], dt.float32)
            nc.sync.dma_start(w1t_sb[:], W1Tst[:])
            alar_sb = cp.tile([F, 2, 8], dt.float32)
            nc.sync.dma_start(alar_sb[:], alar[:])

            psw = pp.tile([F, 8], dt.float32)
            for i in range(2):
                nc.tensor.matmul(psw[:], lhsT=w1t_sb[:, i, :],
                                 rhs=alar_sb[:, i, :], start=(i == 0),
                                 stop=(i == 1))
            nc.vector.tensor_copy(rhs[:, C1:C1 + 8], psw[:])

            for nb in range(NW):
                n0 = nb * WIN
                m = min(WIN, NB - n0)
                ps = pp.tile([WIN, C1 + 8], dt.float32)
                nc.tensor.matmul(ps[:m, :], lhsT=xT_sb[:, n0:n0 + m],
                                 rhs=rhs[:], start=True, stop=True)
                ftile = wp.tile([WIN, C1], dt.bfloat16)
                nc.vector.tensor_copy(ftile[:m, :], ps[:m, 0:C1])
                el_sb = wp.tile([WIN, 4], dt.float32)
                nc.vector.tensor_copy(el_sb[:m, :], ps[:m, C1:C1 + 4])
                er_sb = wp.tile([WIN, 4], dt.float32)
                nc.vector.tensor_copy(er_sb[:m, :], ps[:m, C1 + 4:C1 + 8])
                nc.sync.dma_start(fb[n0:n0 + m, :], ftile[:m, :])
                nc.sync.dma_start(el[n0:n0 + m, :], el_sb[:m, :])
                nc.sync.dma_start(er[n0:n0 + m, :], er_sb[:m, :])
    nc.compile()
    return nc


# ---------------------------------------------------------------------------
# launch B: layer-1 aggregation + h@W2 projection (+ el2/er2)
# ---------------------------------------------------------------------------

def build_launch_B(K_w):
    T = sum(K_w)
    nc = bacc.Bacc("TRN2", target_bir_lowering=False, debug=False,
                   num_devices=CORES)
    table = nc.dram_tensor("table1", [N + 1, C1], dt.bfloat16, kind="ExternalInput")
    offs_src = nc.dram_tensor("offs_src", [WIN, T], dt.int32, kind="ExternalInput")
    el_e = nc.dram_tensor("el_e", [WIN, T, H1], dt.float32, kind="ExternalInput")
    er_e = nc.dram_tensor("er_e", [WIN, T, H1], dt.float32, kind="ExternalInput")
    dstrel = nc.dram_tensor("dstrel", [WIN, T], dt.bfloat16, kind="ExternalInput")
    w2f = nc.dram_tensor("w2f", [F, 2, D], dt.float32, kind="ExternalInput")
    w2b = nc.dram_tensor("w2b", [F, 2, D], dt.bfloat16, kind="ExternalInput")
    al2 = nc.dram_tensor("al2", [WIN, D], dt.float32, kind="ExternalInput")
    ar2 = nc.dram_tensor("ar2", [WIN, D], dt.float32, kind="ExternalInput")
    bias1 = nc.dram_tensor("bias1", [WIN, C1], dt.float32, kind="ExternalInput")
    iota = nc.dram_tensor("iota", [WIN, WIN], dt.bfloat16, kind="ExternalInput")
    ident = nc.dram_tensor("ident", [WIN, WIN], dt.bfloat16, kind="ExternalInput")
    f2x = nc.dram_tensor("f2ext", [NBP, D], dt.bfloat16, kind="ExternalOutput")
    el2 = nc.dram_tensor("el2", [NBP, 1], dt.float32, kind="ExternalOutput")
    er2 = nc.dram_tensor("er2", [NBP, 1], dt.float32, kind="ExternalOutput")

    with tile.TileContext(nc) as tc:
        with tc.tile_pool(name="const", bufs=1) as cp, \
             tc.tile_pool(name="gath", bufs=3) as gp, \
             tc.tile_pool(name="work", bufs=3) as wp, \
             tc.tile_pool(name="psA", bufs=2, space="PSUM") as ppA, \
             tc.tile_pool(name="psT", bufs=2, space="PSUM") as ppT, \
             tc.tile_pool(name="ps2", bufs=2, space="PSUM") as pp2:
            osrc_sb = cp.tile([WIN, T], dt.int32)
            nc.sync.dma_start(osrc_sb[:], offs_src[:])
            el_sb = cp.tile([WIN, T, H1], dt.float32)
            nc.sync.dma_start(el_sb[:], el_e[:])
            er_sb = cp.tile([WIN, T, H1], dt.float32)
            nc.sync.dma_start(er_sb[:], er_e[:])
            drel_sb = cp.tile([WIN, T], dt.bfloat16)
            nc.sync.dma_start(drel_sb[:], dstrel[:])
            iota_sb = cp.tile([WIN, WIN], dt.bfloat16)
            nc.sync.dma_start(iota_sb[:], iota[:])
            ident_sb = cp.tile([WIN, WIN], dt.bfloat16)
            nc.sync.dma_start(ident_sb[:], ident[:])
            bias1_sb = cp.tile([WIN, C1], dt.float32)
            nc.sync.dma_start(bias1_sb[:], bias1[:])
            w2f_sb = cp.tile([F, 2, D], dt.float32)
            nc.sync.dma_start(w2f_sb[:], w2f[:])
            al2_sb = cp.tile([WIN, D], dt.float32)
            nc.sync.dma_start(al2_sb[:], al2[:])
            ar2_sb = cp.tile([WIN, D], dt.float32)
            nc.sync.dma_start(ar2_sb[:], ar2[:])

            # rhs for the h@W2 matmul: [W2 | W2@al2 | W2@ar2] in bf16
            w2e_sb = cp.tile([F, 2, D + 2], dt.bfloat16)
            nc.sync.dma_start(w2e_sb[:, :, 0:D], w2b[:])
            for j, attn_sb in enumerate((al2_sb, ar2_sb)):
                tmp = wp.tile([F, 2, D], dt.float32, tag="w2tmp")
                nc.vector.tensor_tensor(out=tmp[:], in0=w2f_sb[:],
                                        in1=attn_sb[0:F, :].unsqueeze(1)
                                        .broadcast_to([F, 2, D]),
                                        op=mybir.AluOpType.mult)
                red = wp.tile([F, 2, 1], dt.float32, tag="w2red")
                nc.vector.tensor_reduce(red[:], tmp[:],
                                        axis=mybir.AxisListType.X,
                                        op=mybir.AluOpType.add)
                nc.vector.tensor_copy(w2e_sb[:, :, D + j:D + j + 1], red[:])

            t0 = 0
            for w in range(NW):
                K = K_w[w]
                # gathered rows: [feat(256) | ex(4)] bf16 per edge
                gath = gp.tile([WIN, K, C1 + 4], dt.bfloat16, tag="gath")
                for t in range(K):
                    nc.gpsimd.indirect_dma_start(
                        out=gath[:, t, 0:C1], out_offset=None, in_=table[:],
                        in_offset=IndirectOffsetOnAxis(
                            ap=osrc_sb[:, t0 + t:t0 + t + 1], axis=0))
                # scores
                es = wp.tile([WIN, K, H1], dt.float32, tag="es")
                nc.vector.tensor_tensor(out=es[:], in0=el_sb[:, t0:t0 + K, :],
                                        in1=er_sb[:, t0:t0 + K, :],
                                        op=mybir.AluOpType.add)
                _leaky_exp(nc, wp, es[:])
                nc.vector.tensor_copy(gath[:, :, C1:C1 + 4], es[:])
                g4 = gath[:, :, 0:C1].rearrange("p k (h d) -> p k h d", d=D)
                exb = gath[:, :, C1:C1 + 4].unsqueeze(-1) \
                    .broadcast_to([WIN, K, H1, D])
                nc.vector.tensor_tensor(out=g4, in0=g4, in1=exb,
                                        op=mybir.AluOpType.mult)
                # one-hot A (pads have dstrel=-1 -> all-zero row)
                Aw = wp.tile([WIN, K, WIN], dt.bfloat16, tag="Aw")
                i0 = iota_sb[:].unsqueeze(1).broadcast_to([WIN, K, WIN])
                d0 = drel_sb[:, t0:t0 + K].unsqueeze(-1) \
                    .broadcast_to([WIN, K, WIN])
                nc.vector.tensor_tensor(out=Aw[:], in0=i0, in1=d0,
                                        op=mybir.AluOpType.is_equal)
                ps1 = ppA.tile([WIN, C1 + 4], dt.float32)
                for t in range(K):
                    nc.tensor.matmul(ps1[:], lhsT=Aw[:, t, :],
                                     rhs=gath[:, t, :],
                                     start=(t == 0), stop=(t == K - 1))
                # h = relu(S/denom + bias1)
                rec = wp.tile([WIN, H1], dt.float32, tag="rec")
                nc.vector.tensor_scalar_add(rec[:], ps1[:, C1:C1 + 4], EPS)
                nc.vector.reciprocal(rec[:], rec[:])
                h32 = wp.tile([WIN, C1], dt.float32, tag="h32")
                h4 = h32[:].rearrange("p (h d) -> p h d", d=D)
                nc.vector.tensor_tensor(
                    out=h4, in0=ps1[:, 0:C1].rearrange("p (h d) -> p h d", d=D),
                    in1=rec[:].unsqueeze(-1).broadcast_to([WIN, H1, D]),
                    op=mybir.AluOpType.mult)
                nc.vector.tensor_tensor(out=h32[:], in0=h32[:], in1=bias1_sb[:],
                                        op=mybir.AluOpType.add)
                hb = wp.tile([WIN, C1], dt.bfloat16, tag="hb")
                nc.vector.tensor_scalar_max(hb[:], h32[:], 0.0)
                # hT via PE transpose, then feat2 = h @ W2ext
                hT = wp.tile([F, 2, WIN], dt.bfloat16, tag="hT")
                for i in range(2):
                    pst = ppT.tile([WIN, WIN], dt.bfloat16)
                    nc.tensor.transpose(pst[:], hb[:, i * F:(i + 1) * F],
                                        ident_sb[:])
                    nc.vector.tensor_copy(hT[:, i, :], pst[:])
                ps2 = pp2.tile([WIN, D + 2], dt.float32)
                for i in range(2):
                    nc.tensor.matmul(ps2[:], lhsT=hT[:, i, :],
                                     rhs=w2e_sb[:, i, :], start=(i == 0),
                                     stop=(i == 1))
                f2b = wp.tile([WIN, D], dt.bfloat16, tag="f2b")
                nc.vector.tensor_copy(f2b[:], ps2[:, 0:D])
                el2_sb = wp.tile([WIN, 2], dt.float32, tag="el2")
                nc.vector.tensor_copy(el2_sb[:], ps2[:, D:D + 2])
                n0 = w * WIN
                nc.sync.dma_start(f2x[n0:n0 + WIN, :], f2b[:])
                nc.sync.dma_start(el2[n0:n0 + WIN, :], el2_sb[:, 0:1])
                nc.sync.dma_start(er2[n0:n0 + WIN, :], el2_sb[:, 1:2])
                t0 += K
    nc.compile()
    return nc


# ---------------------------------------------------------------------------
# launch C: layer-2 aggregation -> graph_output, graph_embedding
# ---------------------------------------------------------------------------

def build_launch_C(K_w):
    T = sum(K_w)
    nc = bacc.Bacc("TRN2", target_bir_lowering=False, debug=False,
                   num_devices=CORES)
    table = nc.dram_tensor("table2", [N + 1, D], dt.bfloat16, kind="ExternalInput")
    offs_src = nc.dram_tensor("offs_src", [WIN, T], dt.int32, kind="ExternalInput")
    el_e = nc.dram_tensor("el_e", [WIN, T, 1], dt.float32, kind="ExternalInput")
    er_e = nc.dram_tensor("er_e", [WIN, T, 1], dt.float32, kind="ExternalInput")
    dstrel = nc.dram_tensor("dstrel", [WIN, T], dt.bfloat16, kind="ExternalInput")
    bias2 = nc.dram_tensor("bias2", [WIN, D], dt.float32, kind="ExternalInput")
    iota = nc.dram_tensor("iota", [WIN, WIN], dt.bfloat16, kind="ExternalInput")
    gout = nc.dram_tensor("gout", [NBP, D], dt.float32, kind="ExternalOutput")
    gemb = nc.dram_tensor("gemb", [NBP, 1], dt.float32, kind="ExternalOutput")

    with tile.TileContext(nc) as tc:
        with tc.tile_pool(name="const", bufs=1) as cp, \
             tc.tile_pool(name="gath", bufs=3) as gp, \
             tc.tile_pool(name="work", bufs=3) as wp, \
             tc.tile_pool(name="psC", bufs=2, space="PSUM") as ppC:
            osrc_sb = cp.tile([WIN, T], dt.int32)
            nc.sync.dma_start(osrc_sb[:], offs_src[:])
            el_sb = cp.tile([WIN, T, 1], dt.float32)
            nc.sync.dma_start(el_sb[:], el_e[:])
            er_sb = cp.tile([WIN, T, 1], dt.float32)
            nc.sync.dma_start(er_sb[:], er_e[:])
            drel_sb = cp.tile([WIN, T], dt.bfloat16)
            nc.sync.dma_start(drel_sb[:], dstrel[:])
            iota_sb = cp.tile([WIN, WIN], dt.bfloat16)
            nc.sync.dma_start(iota_sb[:], iota[:])
            bias2_sb = cp.tile([WIN, D], dt.float32)
            nc.sync.dma_start(bias2_sb[:], bias2[:])

            t0 = 0
            for w in range(NW):
                K = K_w[w]
                gath = gp.tile([WIN, K, D + 1], dt.bfloat16, tag="gath")
                for t in range(K):
                    nc.gpsimd.indirect_dma_start(
                        out=gath[:, t, 0:D], out_offset=None, in_=table[:],
                        in_offset=IndirectOffsetOnAxis(
                            ap=osrc_sb[:, t0 + t:t0 + t + 1], axis=0))
                es = wp.tile([WIN, K, 1], dt.float32, tag="es")
                nc.vector.tensor_tensor(out=es[:], in0=el_sb[:, t0:t0 + K, :],
                                        in1=er_sb[:, t0:t0 + K, :],
                                        op=mybir.AluOpType.add)
                _leaky_exp(nc, wp, es[:])
                nc.vector.tensor_copy(gath[:, :, D:D + 1], es[:])
                exb = gath[:, :, D:D + 1].broadcast_to([WIN, K, D])
                nc.vector.tensor_tensor(out=gath[:, :, 0:D],
                                        in0=gath[:, :, 0:D], in1=exb,
                                        op=mybir.AluOpType.mult)
                Aw = wp.tile([WIN, K, WIN], dt.bfloat16, tag="Aw")
                i0 = iota_sb[:].unsqueeze(1).broadcast_to([WIN, K, WIN])
                d0 = drel_sb[:, t0:t0 + K].unsqueeze(-1) \
                    .broadcast_to([WIN, K, WIN])
                nc.vector.tensor_tensor(out=Aw[:], in0=i0, in1=d0,
                                        op=mybir.AluOpType.is_equal)
                ps = ppC.tile([WIN, D + 1], dt.float32)
                for t in range(K):
                    nc.tensor.matmul(ps[:], lhsT=Aw[:, t, :],
                                     rhs=gath[:, t, :],
                                     start=(t == 0), stop=(t == K - 1))
                rec = wp.tile([WIN, 1], dt.float32, tag="rec")
                nc.vector.tensor_scalar_add(rec[:], ps[:, D:D + 1], EPS)
                nc.vector.reciprocal(rec[:], rec[:])
                go = wp.tile([WIN, D], dt.float32, tag="go")
                nc.vector.tensor_scalar(out=go[:], in0=ps[:, 0:D],
                                        scalar1=rec[:, 0:1], scalar2=None,
                                        op0=mybir.AluOpType.mult)
                nc.vector.tensor_tensor(out=go[:], in0=go[:], in1=bias2_sb[:],
                                        op=mybir.AluOpType.add)
                nc.vector.tensor_scalar_max(go[:], go[:], 0.0)
                emb = wp.tile([WIN, 1], dt.float32, tag="emb")
                nc.vector.tensor_reduce(emb[:], go[:],
                                        axis=mybir.AxisListType.X,
                                        op=mybir.AluOpType.add)
                n0 = w * WIN
                nc.sync.dma_start(gout[n0:n0 + WIN, :], go[:])
                nc.sync.dma_start(gemb[n0:n0 + WIN, :], emb[:])
                t0 += K
    nc.compile()
    return nc


# ---------------------------------------------------------------------------
# host orchestration
# ---------------------------------------------------------------------------

_cache = {}


def _get_programs(K_w):
    key = tuple(K_w)
    if "A" not in _cache:
        _cache["A"] = build_launch_A()
    if ("B", key) not in _cache:
        _cache[("B", key)] = build_launch_B(K_w)
    if ("C", key) not in _cache:
        _cache[("C", key)] = build_launch_C(K_w)
    return _cache["A"], _cache[("B", key)], _cache[("C", key)]


def kernel(x, src, dst, W1, attn_l1, attn_r1, bias1, W2, attn_l2, attn_r2,
           bias2):
    x = np.ascontiguousarray(np.asarray(x, np.float32))
    src = np.asarray(src, np.int32)
    dst = np.asarray(dst, np.int32)
    W1 = np.asarray(W1, np.float32)
    attn_l1 = np.asarray(attn_l1, np.float32)
    attn_r1 = np.asarray(attn_r1, np.float32)
    bias1 = np.asarray(bias1, np.float32)
    W2 = np.asarray(W2, np.float32)
    attn_l2 = np.asarray(attn_l2, np.float32)
    attn_r2 = np.asarray(attn_r2, np.float32)
    bias2 = np.asarray(bias2, np.float32)

    K_w, per_core = _preprocess_edges(src, dst)
    progA, progB, progC = _get_programs(K_w)
    core_ids = list(range(CORES))

    # ---- launch A ----
    W1Tst = np.ascontiguousarray(W1.T.reshape(2, F, F).transpose(1, 0, 2))
    alar_blk = np.zeros((C1, 8), np.float32)
    for h in range(H1):
        alar_blk[h * D:(h + 1) * D, h] = attn_l1[h]
        alar_blk[h * D:(h + 1) * D, 4 + h] = attn_r1[h]
    alar = np.ascontiguousarray(alar_blk.reshape(2, F, 8).transpose(1, 0, 2))
    in_A = [dict(xT=np.ascontiguousarray(x[c * NB:(c + 1) * NB].T),
                 W1=W1, W1Tst=W1Tst, alar=alar) for c in range(CORES)]
    resA = run_bass_kernel_spmd(progA, in_A, core_ids)

    # ---- host re-shard ----
    table1 = np.zeros((N + 1, C1), BF)
    el1_full = np.zeros((N, H1), np.float32)
    for c in range(CORES):
        table1[c * NB:(c + 1) * NB] = resA.results[c]["featb"][0:NB]
        el1_full[c * NB:(c + 1) * NB] = resA.results[c]["el1"][0:NB]

    iota_np = np.broadcast_to(
        np.arange(WIN, dtype=np.float32).astype(BF)[None, :], (WIN, WIN)).copy()
    ident_np = np.eye(WIN, dtype=np.float32).astype(BF)
    w2f = np.ascontiguousarray(W2.reshape(2, F, D).transpose(1, 0, 2))
    w2b = w2f.astype(BF)
    al2_np = np.broadcast_to(attn_l2.reshape(1, D), (WIN, D)).astype(np.float32).copy()
    ar2_np = np.broadcast_to(attn_r2.reshape(1, D), (WIN, D)).astype(np.float32).copy()
    b1_np = np.broadcast_to(bias1.reshape(1, C1), (WIN, C1)).astype(np.float32).copy()
    in_B = []
    for c in range(CORES):
        pc = per_core[c]
        el_e, er_e = _permute_logits(pc, el1_full,
                                     resA.results[c]["er1"][0:NB])
        in_B.append(dict(
            table1=table1, offs_src=pc["offs_src"], el_e=el_e, er_e=er_e,
            dstrel=pc["dstrel"], w2f=w2f, w2b=w2b, al2=al2_np, ar2=ar2_np,
            bias1=b1_np, iota=iota_np, ident=ident_np))
    resB = run_bass_kernel_spmd(progB, in_B, core_ids)

    # ---- host re-shard ----
    table2 = np.zeros((N + 1, D), BF)
    el2_full = np.zeros((N, 1), np.float32)
    for c in range(CORES):
        table2[c * NB:(c + 1) * NB] = resB.results[c]["f2ext"][0:NB]
        el2_full[c * NB:(c + 1) * NB] = resB.results[c]["el2"][0:NB]

    b2_np = np.broadcast_to(bias2.reshape(1, D), (WIN, D)).astype(np.float32).copy()
    in_C = []
    for c in range(CORES):
        pc = per_core[c]
        el_e, er_e = _permute_logits(pc, el2_full,
                                     resB.results[c]["er2"][0:NB])
        in_C.append(dict(
            table2=table2, offs_src=pc["offs_src"], el_e=el_e, er_e=er_e,
            dstrel=pc["dstrel"], bias2=b2_np, iota=iota_np))
    resC = run_bass_kernel_spmd(progC, in_C, core_ids)

    graph_output = np.concatenate(
        [resC.results[c]["gout"][0:NB] for c in range(CORES)], axis=0)
    graph_embedding = np.concatenate(
        [resC.results[c]["gemb"][0:NB, 0] for c in range(CORES)], axis=0)
    return (graph_embedding.astype(np.float32),
            np.ascontiguousarray(graph_output.astype(np.float32)))
